# revision 19
# baseline (speedup 1.0000x reference)
"""DLinear Trainium2 kernel (nn_DLinear_45990509805636) — v5.

Math: with T=17 and KERNEL_SIZE=37 (PAD=18), every moving-average window
covers the whole sequence plus replicated edges, so

    trend[b,t,:] = (S + (18-t)*x0 + (t+2)*x16) / 37,   S = sum_t x[:,t,:]
    out = x_t @ Ws[t] + trend_raw_t @ Wd[t] + bias[t],
    Wd = (Wt - Ws)/37 (host-folded), trend_raw_t = P + t*Q,
    P = S + 18*x0 + 2*x16, Q = x16 - x0.

v5: P and Q are computed ON HOST (f32, cast bf16) and shipped as a 1MB
per-core input, like the host-folded Wd and the host bias epilogue. This
removes the device S-chain entirely: trend(t) is one DVE STT from P,Q,
so EVERY token can run as a uniform 8-matmul PSUM group
(4x x@Ws + 4x trend@Wd accumulated in one bank) paced only by the PE.
Per-token DMA demand drops to ~1.5MB per 6.8us of matmuls (~220 GB/s
vs 358 available), so after the ramp the schedule is purely PE-bound at
544 N=512 bf16 matmuls (~118us).

Schedule (measured ~135.3us, body fully gapless at the 216ns/MM pacing):
  - ramp loads ride TWO HWDGE rings (x0..x4 + P,Q on ACT -- all dispatched
    before the first ACT drain exists; everything else one JIT-ordered SP
    stream), so per-transfer completion receipts (~1.5-3us lag behind the
    wire under load) overlap across rings.
  - a warm-up burst of zero-matmuls (memset SBUF operands, no DMA deps)
    issues right after the engine preamble, so the HAM clock-gate hits
    8/8 by the time the first real matmul's operands land (~12us).
  - tokens 0..PRE-1 are phase-split (x@Ws parked as f16 via ACT, trend
    part joins later via one DVE STT per (t,j)) to cover the first ~28us
    while P,Q and wd stream in; all other tokens are single 8-MM groups
    drained by ACT straight to the per-token f16 output tile.
  - B-token stores are deferred by two steady tokens so their cross-engine
    combine-waits can never head-of-line block PSUM-drain ACTIVATEs in
    the ACT FIFO (the failure mode that cost 3-28us in earlier revs).
  - output dram is [JB, 128, T, D] so one DMA per token stores all four
    j-tiles (host reshapes back); the final token stores per-j slice
    right after each drain, split across the SP and ACT rings.
  - bias is added on host during the f16->f32 upcast.
  - NOTE: the environment occasionally runs the whole compute-clock domain
    at 5/6 speed for a NEFF load (MM spacing 259ns instead of 216); if a
    measurement looks 1.2x slow, re-run and check MM pacing in the trace.

Sharding: data-parallel over batch, 8 cores x 512 rows; weights + P,Q
replicated/sliced per core.
"""

import sys

sys.path.insert(0, "/opt/trn_rl_repo")

import numpy as np
import ml_dtypes

from concourse import bacc
import concourse.mybir as mybir
import concourse.tile as tile
from concourse.bass_utils import run_bass_kernel_spmd

dt = mybir.dt

B, T, C, D = 4096, 17, 512, 512
NCORES = 8
BC = B // NCORES          # 512 batch rows per core
KC = C // 128             # 4 contraction chunks
JB = BC // 128            # 4 output-row tiles per core

PRE = 5                   # phase-split prologue tokens
WARM = 7                  # zero-matmuls to warm the HAM clock gate before
                          # the first DMA-dependent matmul


def build():
    idt = dt.bfloat16
    nc = bacc.Bacc(None, target_bir_lowering=False, name="dlinear_v5")
    xt = nc.dram_tensor("xt", [T, 128, KC, BC], idt, kind="ExternalInput")
    pqt = nc.dram_tensor("pqt", [2, 128, KC, BC], idt, kind="ExternalInput")
    wst = nc.dram_tensor("wst", [T, 128, KC, D], idt, kind="ExternalInput")
    wdt = nc.dram_tensor("wdt", [T, 128, KC, D], idt, kind="ExternalInput")
    out = nc.dram_tensor("out", [JB, 128, T, D], dt.float16,
                         kind="ExternalOutput")

    with tile.TileContext(nc) as tc:
        with (
            tc.tile_pool(name="xres", bufs=1) as xres,
            tc.tile_pool(name="stats", bufs=1) as stats,
            tc.tile_pool(name="wsbuf", bufs=8) as wsbuf,
            tc.tile_pool(name="wdbuf", bufs=12) as wdbuf,
            tc.tile_pool(name="tbuf", bufs=3) as tbuf,
            tc.tile_pool(name="obuf", bufs=8) as obuf,
            tc.tile_pool(name="psum", bufs=8, space="PSUM") as psum,
        ):
            xsb = xres.tile([128, T, KC, BC], idt)
            pq = stats.tile([128, 2, KC, BC], idt)
            zw = stats.tile([128, 640], idt)

            # HAM warm-up: zero-matmuls with no DMA dependency, issued
            # right after the engine preamble so the PE is at 2.4GHz when
            # the first real matmul's operands land (~11us).
            nc.vector.memset(zw, 0.0)
            wps = psum.tile([128, D], dt.float32, tag="ps", name="warm")
            for i in range(WARM):
                nc.tensor.matmul(wps, zw[:, 0:128], zw[:, 128:640],
                                 start=(i == 0), stop=(i == WARM - 1))

            # ---- DMA: weights on the SP queue, x + P,Q on the ACT queue
            ws_tiles, wd_tiles = {}, {}

            def load_ws(t, split=False):
                w = wsbuf.tile([128, KC, D], idt, tag="ws", name="ws")
                if split:
                    nc.sync.dma_start(w[:, 0:2], wst[t, :, 0:2])
                    nc.sync.dma_start(w[:, 2:4], wst[t, :, 2:4])
                else:
                    nc.sync.dma_start(w, wst[t])
                ws_tiles[t] = w

            def load_wd(t):
                w = wdbuf.tile([128, KC, D], idt, tag="wd", name="wd")
                nc.sync.dma_start(w, wdt[t])
                wd_tiles[t] = w

            def load_x(t0, t1, eng=None):
                (eng or nc.sync).dma_start(
                    xsb[:, t0:t1],
                    xt[t0:t1].rearrange("t p k b -> p t k b"),
                )

            def load_pq():
                nc.scalar.dma_start(
                    pq, pqt.rearrange("s p k b -> p s k b"))

            # Ramp loads ride BOTH HWDGE rings so per-transfer completion
            # receipts (the ~1.5-3us lag behind the wire under load) overlap
            # across rings: the ACT ring carries ONLY x0..x4 + P,Q -- all
            # dispatched before the first ACT drain exists, and few enough
            # that no DMA sem-lane reuse wait can block the ACT queue.
            # Everything else is a single JIT-ordered SP stream.
            load_ws(0)
            load_x(0, 1, nc.scalar)
            load_x(1, 2, nc.scalar)
            load_ws(1)
            load_x(2, 3, nc.scalar)
            load_ws(2)
            load_x(3, 4, nc.scalar)
            load_ws(3)
            load_x(4, 5, nc.scalar)
            load_ws(4)
            load_pq()
            load_wd(0)
            load_wd(1)
            load_wd(2)
            load_x(5, 6)
            load_ws(5)
            load_wd(5)
            load_x(6, 7)
            load_ws(6)
            load_wd(6)
            load_wd(3)
            load_x(7, 9)
            load_ws(7)
            load_wd(7)
            load_wd(4)
            load_x(9, 11)
            load_ws(8)
            load_wd(8)
            load_x(11, 13)
            load_ws(9)
            load_wd(9)
            load_x(13, 15)
            load_ws(10)
            load_wd(10)
            load_x(15, 17)
            for t in range(11, T):
                load_ws(t)
                load_wd(t)

            # ---- per-token f16 output tiles; one store per token
            osb_tiles = {}

            def tok_tile(t):
                if t not in osb_tiles:
                    osb_tiles[t] = obuf.tile(
                        [128, JB, 1, D], dt.float16, tag="osb", name="osb")
                return osb_tiles[t]

            def store_tok(t):
                nc.scalar.dma_start(
                    out[:, :, t:t + 1, :].rearrange("j p t d -> p j t d"),
                    osb_tiles.pop(t))

            def store_tok_slice(t, j):
                # final token: per-j stores on alternating queues, emitted
                # right after each j's drain, so the dispatches and
                # receipts overlap in the drain tail
                eng = nc.scalar if j % 2 == 0 else nc.sync
                eng.dma_start(
                    out[j:j + 1, :, t:t + 1, :].rearrange(
                        "j p t d -> p j t d"),
                    osb_tiles[t][:, j:j + 1])

            def make_trend(t):
                if t == 0:
                    return pq[:, 0]
                trend = tbuf.tile([128, KC, BC], idt, tag="trend",
                                  name="trend")
                nc.vector.scalar_tensor_tensor(
                    trend[:], pq[:, 1], float(t), pq[:, 0],
                    mybir.AluOpType.mult, mybir.AluOpType.add)
                return trend

            def emit_a(t):
                # prologue: x@Ws only, parked as f16
                tile_ = tok_tile(t)
                for j in range(JB):
                    psa = psum.tile([128, D], dt.float32, tag="ps",
                                    name="psa")
                    for k in range(KC):
                        nc.tensor.matmul(
                            psa, xsb[:, t, k, j * 128:(j + 1) * 128],
                            ws_tiles[t][:, k],
                            start=(k == 0), stop=(k == KC - 1),
                        )
                    nc.scalar.copy(tile_[:, j, 0], psa)

            pending_b_store = []
            ready_b_store = []

            def emit_b(t):
                # trend@Wd joins the parked x@Ws part in place (DVE STT).
                # The store is DEFERRED by TWO steady tokens: dispatched
                # from ACT then, its combine-wait is long satisfied, so it
                # can't head-of-line block the PSUM-drain ACTIVATEs behind
                # it in the ACT FIFO (one-token deferral is not enough for
                # a B token immediately preceding the steady, e.g. B4->s8).
                trend = make_trend(t)
                tile_ = tok_tile(t)
                for j in range(JB):
                    psb = psum.tile([128, D], dt.float32, tag="ps",
                                    name="psb")
                    for k in range(KC):
                        nc.tensor.matmul(
                            psb, trend[:, k, j * 128:(j + 1) * 128],
                            wd_tiles[t][:, k],
                            start=(k == 0), stop=(k == KC - 1),
                        )
                    nc.vector.scalar_tensor_tensor(
                        tile_[:, j, 0], psb, 1.0, tile_[:, j, 0],
                        mybir.AluOpType.mult, mybir.AluOpType.add,
                    )
                pending_b_store.append(t)

            def emit_steady(t, last=False):
                # one 8-MM group per (t, j), ACT drains straight to f16
                trend = make_trend(t)
                tile_ = tok_tile(t)
                for j in range(JB):
                    ps = psum.tile([128, D], dt.float32, tag="ps",
                                   name="ps")
                    for k in range(KC):
                        nc.tensor.matmul(
                            ps, xsb[:, t, k, j * 128:(j + 1) * 128],
                            ws_tiles[t][:, k],
                            start=(k == 0), stop=False,
                        )
                    for k in range(KC):
                        nc.tensor.matmul(
                            ps, trend[:, k, j * 128:(j + 1) * 128],
                            wd_tiles[t][:, k],
                            start=False, stop=(k == KC - 1),
                        )
                    nc.scalar.copy(tile_[:, j, 0], ps)
                    if last:
                        store_tok_slice(t, j)
                if last:
                    osb_tiles.pop(t)
                else:
                    while ready_b_store:
                        store_tok(ready_b_store.pop(0))
                    ready_b_store.extend(pending_b_store)
                    pending_b_store.clear()
                    store_tok(t)

            for t in range(PRE):
                emit_a(t)
            sched = [("B", 0), ("B", 1), ("s", 5), ("B", 2), ("s", 6),
                     ("B", 3), ("s", 7), ("B", 4), ("s", 8)]
            sched += [("s", t) for t in range(9, T)]
            for kind, t in sched:
                if kind == "B":
                    emit_b(t)
                else:
                    emit_steady(t, last=(t == T - 1))
    nc.compile()
    return nc


_NC_CACHE = {}


def _get_nc(mode="bf16"):
    if "nc" not in _NC_CACHE:
        _NC_CACHE["nc"] = build()
    return _NC_CACHE["nc"]


MODE = "bf16"


def kernel(x, W_seasonal, b_seasonal, W_trend, b_trend, _trace=False):
    npdt = ml_dtypes.bfloat16
    nc = _get_nc()

    def to_tpkd(w):  # [T, D, C] -> [T, 128, KC, D] (c-major on partitions)
        wt = w.transpose(0, 2, 1).reshape(T, KC, 128, D)
        return np.ascontiguousarray(wt.transpose(0, 2, 1, 3))

    wst = to_tpkd(W_seasonal).astype(npdt)
    wdt = to_tpkd((W_trend - W_seasonal) / 37.0).astype(npdt)
    bias = (b_seasonal + b_trend).astype(np.float32)  # host epilogue

    # trend components (f32 on host, cast bf16): trend_raw_t = P + t*Q
    S = x.sum(axis=1, dtype=np.float64).astype(np.float32)    # [B, C]
    P = S + 18.0 * x[:, 0, :] + 2.0 * x[:, 16, :]
    Q = x[:, 16, :] - x[:, 0, :]

    def to_pkb(v):  # [BC, C] -> [128, KC, BC]
        vt = v.T.reshape(KC, 128, BC)                          # [KC,128,BC]
        return np.ascontiguousarray(vt.transpose(1, 0, 2))

    in_maps = []
    for i in range(NCORES):
        sl = slice(i * BC, (i + 1) * BC)
        xs = x[sl]                                             # [BC, T, C]
        xti = xs.transpose(1, 2, 0).reshape(T, KC, 128, BC)
        xti = np.ascontiguousarray(xti.transpose(0, 2, 1, 3)).astype(npdt)
        pqi = np.stack([to_pkb(P[sl]), to_pkb(Q[sl])]).astype(npdt)
        in_maps.append({"xt": xti, "pqt": pqi, "wst": wst, "wdt": wdt})

    res = run_bass_kernel_spmd(
        nc, in_maps, core_ids=list(range(NCORES)), trace=_trace
    )
    outp = np.concatenate(
        [r["out"].reshape(BC, T, D) for r in res.results], axis=0)
    outp = outp.astype(np.float32)
    outp += bias[None]
    if _trace:
        return outp, res
    return outp


if __name__ == "__main__":
    rng = np.random.default_rng(0)
    x = rng.standard_normal((B, T, C), dtype=np.float32)
    Ws = rng.uniform(-0.04, 0.04, (T, D, C)).astype(np.float32)
    Wt = rng.uniform(-0.04, 0.04, (T, D, C)).astype(np.float32)
    bs = rng.uniform(-0.04, 0.04, (T, D)).astype(np.float32)
    bt = rng.uniform(-0.04, 0.04, (T, D)).astype(np.float32)
    o = kernel(x, Ws, bs, Wt, bt)
    print("out shape:", o.shape, o.dtype)


# revision 24
# speedup vs baseline: 1.0621x; 1.0621x over previous
"""DLinear Trainium2 kernel (nn_DLinear_45990509805636) — v5.

Math: with T=17 and KERNEL_SIZE=37 (PAD=18), every moving-average window
covers the whole sequence plus replicated edges, so

    trend[b,t,:] = (S + (18-t)*x0 + (t+2)*x16) / 37,   S = sum_t x[:,t,:]
    out = x_t @ Ws[t] + trend_raw_t @ Wd[t] + bias[t],
    Wd = (Wt - Ws)/37 (host-folded), trend_raw_t = P + t*Q,
    P = S + 18*x0 + 2*x16, Q = x16 - x0.

v5: P and Q are computed ON HOST (f32, cast bf16) and shipped as a 1MB
per-core input, like the host-folded Wd and the host bias epilogue. This
removes the device S-chain entirely: trend(t) is one DVE STT from P,Q,
so EVERY token can run as a uniform 8-matmul PSUM group
(4x x@Ws + 4x trend@Wd accumulated in one bank) paced only by the PE.
Per-token DMA demand drops to ~1.5MB per 6.8us of matmuls (~220 GB/s
vs 358 available), so after the ramp the schedule is purely PE-bound at
544 N=512 bf16 matmuls (~118us).

Schedule (measured ~135.3us, body fully gapless at the 216ns/MM pacing):
  - ramp loads ride TWO HWDGE rings (x0..x4 + P,Q on ACT -- all dispatched
    before the first ACT drain exists; everything else one JIT-ordered SP
    stream), so per-transfer completion receipts (~1.5-3us lag behind the
    wire under load) overlap across rings.
  - a warm-up burst of zero-matmuls (memset SBUF operands, no DMA deps)
    issues right after the engine preamble, so the HAM clock-gate hits
    8/8 by the time the first real matmul's operands land (~12us).
  - tokens 0..PRE-1 are phase-split (x@Ws parked as f16 via ACT, trend
    part joins later via one DVE STT per (t,j)) to cover the first ~28us
    while P,Q and wd stream in; all other tokens are single 8-MM groups
    drained by ACT straight to the per-token f16 output tile.
  - B-token stores dispatch from the SYNC ring: Tile sem-waits are
    program-order counters, so a B store waits on ALL prior DVE ops; on
    the ACT ring that wait head-of-line blocks the PSUM-drain ACTIVATEs
    behind it (a 3-28us PE stall in earlier revs); on SYNC it is harmless.
  - output dram is [JB, 128, T, D] so one DMA per token stores all four
    j-tiles (host reshapes back); the final token stores per-j slice
    right after each drain, split across the SP and ACT rings.
  - bias is added on host during the f16->f32 upcast.
  - NOTE: the environment occasionally runs the whole compute-clock domain
    at 5/6 speed for a NEFF load (MM spacing 259ns instead of 216); if a
    measurement looks 1.2x slow, re-run and check MM pacing in the trace.

Sharding: data-parallel over batch, 8 cores x 512 rows; weights + P,Q
replicated/sliced per core.
"""

import sys

sys.path.insert(0, "/opt/trn_rl_repo")

import numpy as np
import ml_dtypes

from concourse import bacc
import concourse.mybir as mybir
import concourse.tile as tile
from concourse.bass_utils import run_bass_kernel_spmd

dt = mybir.dt

B, T, C, D = 4096, 17, 512, 512
NCORES = 8
BC = B // NCORES          # 512 batch rows per core
KC = C // 128             # 4 contraction chunks
JB = BC // 128            # 4 output-row tiles per core

PRE = 5                   # phase-split prologue tokens
WARM = 7                  # zero-matmuls to warm the HAM clock gate before
                          # the first DMA-dependent matmul


def build():
    idt = dt.bfloat16
    nc = bacc.Bacc(None, target_bir_lowering=False, name="dlinear_v5")
    xt = nc.dram_tensor("xt", [T, 128, KC, BC], idt, kind="ExternalInput")
    pqt = nc.dram_tensor("pqt", [2, 128, KC, BC], idt, kind="ExternalInput")
    wst = nc.dram_tensor("wst", [T, 128, KC, D], idt, kind="ExternalInput")
    wdt = nc.dram_tensor("wdt", [T, 128, KC, D], idt, kind="ExternalInput")
    out = nc.dram_tensor("out", [JB, 128, T, D], dt.float16,
                         kind="ExternalOutput")

    with tile.TileContext(nc) as tc:
        with (
            tc.tile_pool(name="xres", bufs=1) as xres,
            tc.tile_pool(name="stats", bufs=1) as stats,
            tc.tile_pool(name="wsbuf", bufs=8) as wsbuf,
            tc.tile_pool(name="wdbuf", bufs=12) as wdbuf,
            tc.tile_pool(name="tbuf", bufs=3) as tbuf,
            tc.tile_pool(name="obuf", bufs=8) as obuf,
            tc.tile_pool(name="psum", bufs=8, space="PSUM") as psum,
        ):
            xsb = xres.tile([128, T, KC, BC], idt)
            pq = stats.tile([128, 2, KC, BC], idt)
            zw = stats.tile([128, 640], idt)

            # HAM warm-up: zero-matmuls with no DMA dependency, issued
            # right after the engine preamble so the PE is at 2.4GHz when
            # the first real matmul's operands land (~11us).
            nc.vector.memset(zw, 0.0)
            wps = psum.tile([128, D], dt.float32, tag="ps", name="warm")
            for i in range(WARM):
                nc.tensor.matmul(wps, zw[:, 0:128], zw[:, 128:640],
                                 start=(i == 0), stop=(i == WARM - 1))

            # ---- DMA: weights on the SP queue, x + P,Q on the ACT queue
            ws_tiles, wd_tiles = {}, {}

            def load_ws(t, split=False):
                w = wsbuf.tile([128, KC, D], idt, tag="ws", name="ws")
                if split:
                    nc.sync.dma_start(w[:, 0:2], wst[t, :, 0:2])
                    nc.sync.dma_start(w[:, 2:4], wst[t, :, 2:4])
                else:
                    nc.sync.dma_start(w, wst[t])
                ws_tiles[t] = w

            def load_wd(t):
                w = wdbuf.tile([128, KC, D], idt, tag="wd", name="wd")
                nc.sync.dma_start(w, wdt[t])
                wd_tiles[t] = w

            def load_x(t0, t1, eng=None):
                (eng or nc.sync).dma_start(
                    xsb[:, t0:t1],
                    xt[t0:t1].rearrange("t p k b -> p t k b"),
                )

            def load_pq():
                nc.scalar.dma_start(
                    pq, pqt.rearrange("s p k b -> p s k b"))

            # Ramp loads ride BOTH HWDGE rings so per-transfer completion
            # receipts (the ~1.5-3us lag behind the wire under load) overlap
            # across rings: the ACT ring carries ONLY x0..x4 + P,Q -- all
            # dispatched before the first ACT drain exists, and few enough
            # that no DMA sem-lane reuse wait can block the ACT queue.
            # Everything else is a single JIT-ordered SP stream.
            load_ws(0)
            load_x(0, 1, nc.scalar)
            load_x(1, 2, nc.scalar)
            load_ws(1)
            load_x(2, 3, nc.scalar)
            load_ws(2)
            load_x(3, 4, nc.scalar)
            load_ws(3)
            load_x(4, 5, nc.scalar)
            load_ws(4)
            load_pq()
            load_wd(0)
            load_wd(1)
            load_wd(2)
            load_x(5, 6)
            load_ws(5)
            load_wd(5)
            load_x(6, 7)
            load_ws(6)
            load_wd(6)
            load_wd(3)
            load_x(7, 9)
            load_ws(7)
            load_wd(7)
            load_wd(4)
            load_x(9, 11)
            load_ws(8)
            load_wd(8)
            load_x(11, 13)
            load_ws(9)
            load_wd(9)
            load_x(13, 15)
            load_ws(10)
            load_wd(10)
            load_x(15, 17)
            for t in range(11, T):
                load_ws(t)
                load_wd(t)

            # ---- per-token f16 output tiles; one store per token
            osb_tiles = {}

            def tok_tile(t):
                if t not in osb_tiles:
                    osb_tiles[t] = obuf.tile(
                        [128, JB, 1, D], dt.float16, tag="osb", name="osb")
                return osb_tiles[t]

            def store_tok(t, eng=None):
                (eng or nc.scalar).dma_start(
                    out[:, :, t:t + 1, :].rearrange("j p t d -> p j t d"),
                    osb_tiles.pop(t))

            def store_tok_slice(t, j):
                # final token: per-j stores on alternating queues, emitted
                # right after each j's drain, so the dispatches and
                # receipts overlap in the drain tail
                eng = nc.scalar if j % 2 == 0 else nc.sync
                eng.dma_start(
                    out[j:j + 1, :, t:t + 1, :].rearrange(
                        "j p t d -> p j t d"),
                    osb_tiles[t][:, j:j + 1])

            def make_trend(t):
                if t == 0:
                    return pq[:, 0]
                trend = tbuf.tile([128, KC, BC], idt, tag="trend",
                                  name="trend")
                nc.vector.scalar_tensor_tensor(
                    trend[:], pq[:, 1], float(t), pq[:, 0],
                    mybir.AluOpType.mult, mybir.AluOpType.add)
                return trend

            def emit_a(t):
                # prologue: x@Ws only, parked as f16
                tile_ = tok_tile(t)
                for j in range(JB):
                    psa = psum.tile([128, D], dt.float32, tag="ps",
                                    name="psa")
                    for k in range(KC):
                        nc.tensor.matmul(
                            psa, xsb[:, t, k, j * 128:(j + 1) * 128],
                            ws_tiles[t][:, k],
                            start=(k == 0), stop=(k == KC - 1),
                        )
                    nc.scalar.copy(tile_[:, j, 0], psa)

            def emit_b(t):
                # trend@Wd joins the parked x@Ws part in place (DVE STT).
                # The store goes out on the SYNC ring: Tile's sem-waits are
                # program-order counters, so this store waits on every DVE
                # op emitted before it -- on the ACT ring that wait would
                # head-of-line block the PSUM-drain ACTIVATEs behind it
                # (a 3-6us PE stall); on SYNC, whose queue only holds
                # remaining load dispatches, the wait is harmless.
                trend = make_trend(t)
                tile_ = tok_tile(t)
                for j in range(JB):
                    psb = psum.tile([128, D], dt.float32, tag="ps",
                                    name="psb")
                    for k in range(KC):
                        nc.tensor.matmul(
                            psb, trend[:, k, j * 128:(j + 1) * 128],
                            wd_tiles[t][:, k],
                            start=(k == 0), stop=(k == KC - 1),
                        )
                    nc.vector.scalar_tensor_tensor(
                        tile_[:, j, 0], psb, 1.0, tile_[:, j, 0],
                        mybir.AluOpType.mult, mybir.AluOpType.add,
                    )
                store_tok(t, nc.sync)

            def emit_steady(t, last=False):
                # one 8-MM group per (t, j), ACT drains straight to f16
                trend = make_trend(t)
                tile_ = tok_tile(t)
                for j in range(JB):
                    ps = psum.tile([128, D], dt.float32, tag="ps",
                                   name="ps")
                    for k in range(KC):
                        nc.tensor.matmul(
                            ps, xsb[:, t, k, j * 128:(j + 1) * 128],
                            ws_tiles[t][:, k],
                            start=(k == 0), stop=False,
                        )
                    for k in range(KC):
                        nc.tensor.matmul(
                            ps, trend[:, k, j * 128:(j + 1) * 128],
                            wd_tiles[t][:, k],
                            start=False, stop=(k == KC - 1),
                        )
                    nc.scalar.copy(tile_[:, j, 0], ps)
                    if last:
                        store_tok_slice(t, j)
                if last:
                    osb_tiles.pop(t)
                else:
                    store_tok(t)

            for t in range(PRE):
                emit_a(t)
            sched = [("B", 0), ("B", 1), ("s", 5), ("B", 2), ("s", 6),
                     ("B", 3), ("s", 7), ("B", 4), ("s", 8)]
            sched += [("s", t) for t in range(9, T)]
            for kind, t in sched:
                if kind == "B":
                    emit_b(t)
                else:
                    emit_steady(t, last=(t == T - 1))
    nc.compile()
    return nc


_NC_CACHE = {}


def _get_nc(mode="bf16"):
    if "nc" not in _NC_CACHE:
        _NC_CACHE["nc"] = build()
    return _NC_CACHE["nc"]


MODE = "bf16"


def kernel(x, W_seasonal, b_seasonal, W_trend, b_trend, _trace=False):
    npdt = ml_dtypes.bfloat16
    nc = _get_nc()

    def to_tpkd(w):  # [T, D, C] -> [T, 128, KC, D] (c-major on partitions)
        wt = w.transpose(0, 2, 1).reshape(T, KC, 128, D)
        return np.ascontiguousarray(wt.transpose(0, 2, 1, 3))

    wst = to_tpkd(W_seasonal).astype(npdt)
    wdt = to_tpkd((W_trend - W_seasonal) / 37.0).astype(npdt)
    bias = (b_seasonal + b_trend).astype(np.float32)  # host epilogue

    # trend components (f32 on host, cast bf16): trend_raw_t = P + t*Q
    S = x.sum(axis=1, dtype=np.float64).astype(np.float32)    # [B, C]
    P = S + 18.0 * x[:, 0, :] + 2.0 * x[:, 16, :]
    Q = x[:, 16, :] - x[:, 0, :]

    def to_pkb(v):  # [BC, C] -> [128, KC, BC]
        vt = v.T.reshape(KC, 128, BC)                          # [KC,128,BC]
        return np.ascontiguousarray(vt.transpose(1, 0, 2))

    in_maps = []
    for i in range(NCORES):
        sl = slice(i * BC, (i + 1) * BC)
        xs = x[sl]                                             # [BC, T, C]
        xti = xs.transpose(1, 2, 0).reshape(T, KC, 128, BC)
        xti = np.ascontiguousarray(xti.transpose(0, 2, 1, 3)).astype(npdt)
        pqi = np.stack([to_pkb(P[sl]), to_pkb(Q[sl])]).astype(npdt)
        in_maps.append({"xt": xti, "pqt": pqi, "wst": wst, "wdt": wdt})

    res = run_bass_kernel_spmd(
        nc, in_maps, core_ids=list(range(NCORES)), trace=_trace
    )
    outp = np.concatenate(
        [r["out"].reshape(BC, T, D) for r in res.results], axis=0)
    outp = outp.astype(np.float32)
    outp += bias[None]
    if _trace:
        return outp, res
    return outp


if __name__ == "__main__":
    rng = np.random.default_rng(0)
    x = rng.standard_normal((B, T, C), dtype=np.float32)
    Ws = rng.uniform(-0.04, 0.04, (T, D, C)).astype(np.float32)
    Wt = rng.uniform(-0.04, 0.04, (T, D, C)).astype(np.float32)
    bs = rng.uniform(-0.04, 0.04, (T, D)).astype(np.float32)
    bt = rng.uniform(-0.04, 0.04, (T, D)).astype(np.float32)
    o = kernel(x, Ws, bs, Wt, bt)
    print("out shape:", o.shape, o.dtype)
